# revision 11
# baseline (speedup 1.0000x reference)
"""Differentiable 3DGS tile rasterizer forward pass on 8 Trainium2 NeuronCores.

Strategy (sharding_hint: shard pixels, replicate gaussian params):
  Host: depth-sort gaussians, compute conic + per-block (32x32 px) polynomial
  coefficients, cull per block on the alpha >= 1/255 support, then pack
  blocks into 128-row "superchunks": the 128 PE partitions are split into
  four 32-row groups, each group holding (a slice of) one block's gaussian
  list. One block may span 1..4 consecutive groups (a "run").

  Device (SPMD over 8 cores, S superchunks each), per superchunk:
    z[g, p]  = coef_g . basis_p     4x2 packed tile_position matmuls (fp32)
    e        = exp(z)               ScalarE          == op*exp(power)
    m        = e >= 1/255           VectorE
    alpha    = min(e, 0.99) * m     VectorE fused stt
    s        = ln(1 - alpha)        ScalarE, fp16 out
    S[g, p]  = sum_{k<g, same run} s[k, p]   per-superchunk triangular matmul
    T        = exp(S)               ScalarE   exclusive transmittance
    w        = alpha * T            VectorE, fp16 out
    C[q, 12*jc + 3*j0 + c] = sum_g w[g, 128*jc + q] col_bd[g, .]  (matmul)
  Host: scatter per-(superchunk, run) C back into the [3, H, W] image.
"""

import sys

sys.path.insert(0, "/opt/trn_rl_repo")

import numpy as np

P, H, W = 2048, 512, 512
BW = BH = 32                      # pixel block size
NBX, NBY = W // BW, H // BH       # 16 x 16 blocks
NBLOCKS = NBX * NBY               # 256
NCORES = 8
NPIX = BW * BH                    # 1024 pixels per block
CAP = 128                         # max gaussians per block (4 groups x 32)
GRP = 32                          # rows per group
BCW = NPIX + 2 * GRP              # packed [basis | coef_hi | coef_lo] per group
OB = 4                            # superchunks per output DMA batch

_STATE = {}


def _patch_act_tables():
    """Make Exp/Ln resolve only to the combined natural_log_exp_and_others
    table set, so the act-table-load pass emits one load instead of
    alternating ~2.7us set switches between every Exp and Ln activation."""
    from concourse import bacc, mybir, hw_specs

    if getattr(bacc, "_act_tables_patched", False):
        return
    orig = hw_specs.get_activation_tables
    both = {mybir.ActivationFunctionType.Exp, mybir.ActivationFunctionType.Ln}

    def patched(arch):
        tabs = dict(orig(arch))
        return {name: (fns if name == "natural_log_exp_and_others"
                       else set(fns) - both)
                for name, fns in tabs.items()}

    hw_specs.get_activation_tables = patched
    bacc.get_activation_tables = patched
    bacc._act_tables_patched = True


def _build_module(S):
    import concourse.tile as tile
    from concourse import bacc, mybir
    from contextlib import ExitStack

    _patch_act_tables()

    fp32 = mybir.dt.float32
    fp16 = mybir.dt.float16
    Act = mybir.ActivationFunctionType
    Alu = mybir.AluOpType

    nc = bacc.Bacc("TRN2", target_bir_lowering=False, debug=False,
                   num_devices=NCORES)

    bc_ap = nc.dram_tensor("bc", [S, 4, 6, BCW], fp16,
                           kind="ExternalInput").ap()
    col_ap = nc.dram_tensor("colors", [CAP, S * 12], fp16,
                            kind="ExternalInput").ap()
    u_ap = nc.dram_tensor("u", [CAP, S * CAP], fp16,
                          kind="ExternalInput").ap()
    out_ap = nc.dram_tensor("outC", [128, S * 96], fp32,
                            kind="ExternalOutput").ap()

    with tile.TileContext(nc) as tc:
        with ExitStack() as ctx:
            up = ctx.enter_context(tc.tile_pool(name="u", bufs=1))
            lp = ctx.enter_context(tc.tile_pool(name="col", bufs=1))
            bp = ctx.enter_context(tc.tile_pool(name="bc", bufs=3))
            ep = ctx.enter_context(tc.tile_pool(name="e", bufs=2))
            mp = ctx.enter_context(tc.tile_pool(name="m", bufs=2))
            ap_ = ctx.enter_context(tc.tile_pool(name="alpha", bufs=2))
            sp = ctx.enter_context(tc.tile_pool(name="s", bufs=2))
            tp = ctx.enter_context(tc.tile_pool(name="t", bufs=2))
            wp = ctx.enter_context(tc.tile_pool(name="w", bufs=2))
            cop = ctx.enter_context(tc.tile_pool(name="cout", bufs=2))
            zp = ctx.enter_context(tc.tile_pool(name="z", bufs=2, space="PSUM"))
            Sp = ctx.enter_context(tc.tile_pool(name="S", bufs=1, space="PSUM"))
            Cp = ctx.enter_context(tc.tile_pool(name="C", bufs=2, space="PSUM"))

            u_all = up.tile([CAP, S * CAP], fp16)
            nc.sync.dma_start(u_all[:], u_ap[:])
            col_all = lp.tile([CAP, S * 12], fp16)
            nc.sync.dma_start(col_all[:], col_ap[:])

            ostage = None
            for s in range(S):
                bcp_t = bp.tile([128, BCW], fp16)
                engines = [nc.sync, nc.scalar, nc.sync, nc.scalar]
                for j in range(4):
                    engines[j].dma_start(bcp_t[GRP * j:GRP * j + 6, :],
                                         bc_ap[s, j])

                z_t = zp.tile([128, NPIX], fp32)
                for j in range(4):
                    for h in range(2):
                        for pp in range(2):  # coef hi then lo, accumulated
                            nc.tensor.matmul(
                                z_t[GRP * j:GRP * (j + 1),
                                    h * 512:(h + 1) * 512],
                                bcp_t[GRP * j:GRP * j + 6,
                                      NPIX + GRP * pp:NPIX + GRP * (pp + 1)],
                                bcp_t[GRP * j:GRP * j + 6,
                                      h * 512:(h + 1) * 512],
                                start=(pp == 0), stop=(pp == 1),
                                tile_position=(GRP * j, GRP * j))

                e_t = ep.tile([128, NPIX], fp32)
                nc.scalar.activation(e_t[:], z_t[:], Act.Exp)

                m_t = mp.tile([128, NPIX], fp32)
                nc.vector.tensor_scalar(m_t[:], e_t[:], 1.0 / 255.0, None,
                                        Alu.is_ge)

                al_t = ap_.tile([128, NPIX], fp32)
                nc.vector.scalar_tensor_tensor(al_t[:], e_t[:], 0.99, m_t[:],
                                               Alu.min, Alu.mult)

                s_t = sp.tile([128, NPIX], fp16)
                nc.scalar.activation(s_t[:], al_t[:], Act.Ln, bias=1.0,
                                     scale=-1.0)

                S_t = Sp.tile([128, NPIX], fp32)
                for h in range(2):
                    nc.tensor.matmul(S_t[:, h * 512:(h + 1) * 512],
                                     u_all[:, s * CAP:(s + 1) * CAP],
                                     s_t[:, h * 512:(h + 1) * 512],
                                     start=True, stop=True)

                T_t = tp.tile([128, NPIX], fp32)
                nc.scalar.activation(T_t[:], S_t[:], Act.Exp)

                w_t = wp.tile([128, NPIX], fp16)
                nc.vector.tensor_tensor(w_t[:], al_t[:], T_t[:], Alu.mult)

                C_t = Cp.tile([128, 96], fp32)
                for jc in range(8):
                    nc.tensor.matmul(C_t[:, jc * 12:(jc + 1) * 12],
                                     w_t[:, jc * 128:(jc + 1) * 128],
                                     col_all[:, s * 12:(s + 1) * 12],
                                     start=True, stop=True)

                g = s % OB
                if g == 0:
                    ostage = cop.tile([128, OB * 96], fp32)
                nc.vector.tensor_copy(ostage[:, g * 96:(g + 1) * 96], C_t[:])
                if g == OB - 1 or s == S - 1:
                    s0 = s - g
                    nc.scalar.dma_start(
                        out_ap[:, s0 * 96:(s + 1) * 96],
                        ostage[:, :(g + 1) * 96])

    nc.compile()
    return nc


def _get_state(S):
    key = ("nc", S)
    if key not in _STATE:
        _STATE[key] = _build_module(S)
    return _STATE[key]


def _prepare_inputs(means_2d, covs_2d, depth_features, opacity_features,
                    color_features):
    """Host prep: sort, conic, per-block cull, superchunk packing.

    Returns (in_maps, S, block_map) where block_map[bidx] =
    (core, superchunk, j0) for every scheduled (non-empty) block.
    """
    order = np.argsort(depth_features[:, 0], kind="stable")
    m = means_2d[order].astype(np.float64)
    cv = covs_2d[order].astype(np.float64)
    op = opacity_features[order, 0].astype(np.float64)
    col = color_features[order].astype(np.float64)

    a, b, c = cv[:, 0], cv[:, 1], cv[:, 2]
    det = np.maximum(a * c - b * b, 1e-8)
    ia, ib, ic = c / det, -b / det, a / det

    alive = op * 255.0 >= 1.0 - 1e-6
    qsel = np.where(alive, 2.0 * np.log(np.maximum(255.0 * op, 1.0)), 0.0) + 0.3
    dx = np.sqrt(np.maximum(qsel * a, 0.0)) + 0.5
    dy = np.sqrt(np.maximum(qsel * c, 0.0)) + 0.5

    mx, my = m[:, 0], m[:, 1]
    bx0 = np.arange(NBX) * BW
    by0 = np.arange(NBY) * BH
    selx = (mx[:, None] + dx[:, None] >= bx0[None, :] + 0.5) & \
           (mx[:, None] - dx[:, None] <= bx0[None, :] + BW - 0.5)
    sely = (my[:, None] + dy[:, None] >= by0[None, :] + 0.5) & \
           (my[:, None] - dy[:, None] <= by0[None, :] + BH - 0.5)
    sel = selx[:, None, :] & sely[:, :, None] & alive[:, None, None]

    # block lists (depth order preserved: np.nonzero is ascending)
    blocks = []  # (bidx, idx array, ngroups)
    for byi in range(NBY):
        for bxi in range(NBX):
            bidx = byi * NBX + bxi
            idx = np.nonzero(sel[:, byi, bxi])[0]
            L = idx.size
            if L == 0:
                continue
            if L > CAP:
                raise RuntimeError(f"block {bidx}: {L} gaussians > {CAP}")
            blocks.append((bidx, idx, (L + GRP - 1) // GRP))

    # assign blocks to cores balancing total group count
    blocks.sort(key=lambda t: (-t[2], -t[1].size))
    core_groups = [0] * NCORES
    core_blocks = [[] for _ in range(NCORES)]
    for blk in blocks:
        ci = min(range(NCORES), key=lambda cc: core_groups[cc])
        core_blocks[ci].append(blk)
        core_groups[ci] += blk[2]

    # pack each core's blocks into superchunks (first-fit decreasing;
    # each block occupies ng consecutive groups of one superchunk)
    core_scs = []
    for ci in range(NCORES):
        scs = []   # each: list of (bidx, idx, j0, ng)
        free = []  # free groups per superchunk
        for bidx, idx, ng in core_blocks[ci]:
            for si, fr in enumerate(free):
                if fr >= ng:
                    j0 = 4 - fr
                    scs[si].append((bidx, idx, j0, ng))
                    free[si] -= ng
                    break
            else:
                scs.append([(bidx, idx, 0, ng)])
                free.append(4 - ng)
        core_scs.append(scs)

    S = max(len(scs) for scs in core_scs)

    # packed arrays
    ixl = np.arange(BW, dtype=np.float64) + 0.5 - BW / 2
    iyl = np.arange(BH, dtype=np.float64) + 0.5 - BH / 2
    Xl = np.tile(ixl, BH)               # pixel p = iy*BW + ix
    Yl = np.repeat(iyl, BW)
    basis_block = np.stack(
        [np.ones(NPIX), Xl, Yl, Xl * Xl, Xl * Yl, Yl * Yl]).astype(np.float16)
    basis_dummy = np.zeros((6, NPIX), np.float16)
    basis_dummy[0] = 1.0

    in_maps = []
    block_map = {}
    for ci in range(NCORES):
        bc = np.zeros((S, 4, 6, BCW), np.float16)
        bc[:, :, :, :NPIX] = basis_dummy[None, None]
        bc[:, :, 0, NPIX:NPIX + GRP] = -30000.0
        colbd = np.zeros((CAP, S, 12), np.float16)
        u = np.zeros((CAP, S, CAP), np.float16)
        for si, sc in enumerate(core_scs[ci]):
            for bidx, idx, j0, ng in sc:
                byi, bxi = divmod(bidx, NBX)
                cx = bx0[bxi] + BW / 2
                cy = by0[byi] + BH / 2
                L = idx.size
                mxp = mx[idx] - cx
                myp = my[idx] - cy
                cf = np.zeros((6, L))
                cf[0] = (-0.5 * ia[idx] * mxp * mxp - ib[idx] * mxp * myp
                         - 0.5 * ic[idx] * myp * myp + np.log(op[idx]))
                cf[1] = ia[idx] * mxp + ib[idx] * myp
                cf[2] = ib[idx] * mxp + ic[idx] * myp
                cf[3] = -0.5 * ia[idx]
                cf[4] = -ib[idx]
                cf[5] = -0.5 * ic[idx]
                cf = cf.astype(np.float32)
                cf_hi = cf.astype(np.float16)
                cf_lo = (cf - cf_hi.astype(np.float32)).astype(np.float16)
                for li in range(ng):
                    j = j0 + li
                    lo = li * GRP
                    n = min(GRP, L - lo)
                    bc[si, j, :, :NPIX] = basis_block
                    bc[si, j, :, NPIX:NPIX + n] = cf_hi[:, lo:lo + n]
                    bc[si, j, :, NPIX + GRP:NPIX + GRP + n] = \
                        cf_lo[:, lo:lo + n]
                r0, r1 = GRP * j0, GRP * j0 + L
                colbd[r0:r1, si, 3 * j0:3 * j0 + 3] = \
                    col[idx].astype(np.float16)
                u[r0:r1, si, r0:r1] = np.triu(np.ones((L, L), np.float16), 1)
                block_map[bidx] = (ci, si, j0)
        in_maps.append({
            "bc": bc,
            "colors": np.ascontiguousarray(colbd.reshape(CAP, S * 12)),
            "u": np.ascontiguousarray(u.reshape(CAP, S * CAP)),
        })
    return in_maps, S, block_map


def _unshard(results, S, block_map):
    out = np.zeros((3, H, W), np.float32)
    for bidx, (ci, si, j0) in block_map.items():
        byi, bxi = divmod(bidx, NBX)
        Cc = results[ci]["outC"]  # [128, S*96]
        blk = Cc[:, si * 96:(si + 1) * 96].reshape(128, 8, 12)
        # C[ch, 128*jc + q] = blk[q, jc, 3*j0 + ch]
        cb = blk[:, :, 3 * j0:3 * j0 + 3].transpose(2, 1, 0).reshape(3, NPIX)
        out[:, byi * BH:(byi + 1) * BH, bxi * BW:(bxi + 1) * BW] = \
            cb.reshape(3, BH, BW)
    return out


def kernel(means_2d, covs_2d, depth_features, opacity_features,
           color_features, screen_space_points=None, width=W, height=H,
           **_unused):
    from concourse.bass_utils import run_bass_kernel_spmd

    in_maps, S, block_map = _prepare_inputs(
        np.asarray(means_2d), np.asarray(covs_2d),
        np.asarray(depth_features), np.asarray(opacity_features),
        np.asarray(color_features))
    nc = _get_state(S)
    res = run_bass_kernel_spmd(nc, in_maps, core_ids=list(range(NCORES)))
    return _unshard(res.results, S, block_map)


# revision 12
# speedup vs baseline: 6.2216x; 6.2216x over previous
"""Differentiable 3DGS tile rasterizer forward pass on 8 Trainium2 NeuronCores.

Strategy (sharding_hint: shard pixels, replicate gaussian params):
  Host: depth-sort gaussians, compute conic + per-block (32x32 px) polynomial
  coefficients, cull per block on the alpha >= 1/255 support, then pack
  blocks into 128-row "superchunks": the 128 PE partitions are split into
  four 32-row groups, each group holding (a slice of) one block's gaussian
  list. One block may span 1..4 consecutive groups (a "run").

  Device (SPMD over 8 cores, S superchunks each), per superchunk:
    z[g, p]  = coef_g . basis_p     4x2 packed tile_position matmuls (fp32)
    e        = exp(z)               ScalarE          == op*exp(power)
    m        = e >= 1/255           VectorE
    alpha    = min(e, 0.99) * m     VectorE fused stt
    s        = ln(1 - alpha)        ScalarE, fp16 out
    S[g, p]  = sum_{k<g, same run} s[k, p]   per-superchunk triangular matmul
    T        = exp(S)               ScalarE   exclusive transmittance
    w        = alpha * T            VectorE, fp16 out
    C[q, 12*jc + 3*j0 + c] = sum_g w[g, 128*jc + q] col_bd[g, .]  (matmul)
  Host: scatter per-(superchunk, run) C back into the [3, H, W] image.
"""

import sys

sys.path.insert(0, "/opt/trn_rl_repo")

import numpy as np

P, H, W = 2048, 512, 512
BW = BH = 32                      # pixel block size
NBX, NBY = W // BW, H // BH       # 16 x 16 blocks
NBLOCKS = NBX * NBY               # 256
NCORES = 8
NPIX = BW * BH                    # 1024 pixels per block
CAP = 128                         # max gaussians per block (4 groups x 32)
GRP = 32                          # rows per group
BCW = NPIX + 2 * GRP              # packed [basis | coef_hi | coef_lo] per group
OB = 4                            # superchunks per output DMA batch

_STATE = {}


def _patch_act_tables():
    """Make Exp/Ln resolve only to the combined natural_log_exp_and_others
    table set, so the act-table-load pass emits one load instead of
    alternating ~2.7us set switches between every Exp and Ln activation."""
    from concourse import bacc, mybir, hw_specs

    if getattr(bacc, "_act_tables_patched", False):
        return
    orig = hw_specs.get_activation_tables
    both = {mybir.ActivationFunctionType.Exp, mybir.ActivationFunctionType.Ln}

    def patched(arch):
        tabs = dict(orig(arch))
        return {name: (fns if name == "natural_log_exp_and_others"
                       else set(fns) - both)
                for name, fns in tabs.items()}

    hw_specs.get_activation_tables = patched
    bacc.get_activation_tables = patched
    bacc._act_tables_patched = True


def _build_module(S):
    import concourse.tile as tile
    from concourse import bacc, mybir
    from contextlib import ExitStack

    _patch_act_tables()

    fp32 = mybir.dt.float32
    fp16 = mybir.dt.float16
    Act = mybir.ActivationFunctionType
    Alu = mybir.AluOpType

    nc = bacc.Bacc("TRN2", target_bir_lowering=False, debug=False,
                   num_devices=NCORES)

    bc_ap = nc.dram_tensor("bc", [4, 6, S * BCW], fp16,
                           kind="ExternalInput").ap()
    col_ap = nc.dram_tensor("colors", [CAP, S * 12], fp16,
                            kind="ExternalInput").ap()
    u_ap = nc.dram_tensor("u", [CAP, S * CAP], fp16,
                          kind="ExternalInput").ap()
    out_ap = nc.dram_tensor("outC", [128, S * 96], fp32,
                            kind="ExternalOutput").ap()

    with tile.TileContext(nc) as tc:
        with ExitStack() as ctx:
            up = ctx.enter_context(tc.tile_pool(name="u", bufs=1))
            lp = ctx.enter_context(tc.tile_pool(name="col", bufs=1))
            bp = ctx.enter_context(tc.tile_pool(name="bc", bufs=1))
            ep = ctx.enter_context(tc.tile_pool(name="e", bufs=2))
            mp = ctx.enter_context(tc.tile_pool(name="m", bufs=2))
            ap_ = ctx.enter_context(tc.tile_pool(name="alpha", bufs=2))
            sp = ctx.enter_context(tc.tile_pool(name="s", bufs=2))
            tp = ctx.enter_context(tc.tile_pool(name="t", bufs=2))
            wp = ctx.enter_context(tc.tile_pool(name="w", bufs=2))
            cop = ctx.enter_context(tc.tile_pool(name="cout", bufs=2))
            zp = ctx.enter_context(tc.tile_pool(name="z", bufs=2, space="PSUM"))
            Sp = ctx.enter_context(tc.tile_pool(name="S", bufs=1, space="PSUM"))
            Cp = ctx.enter_context(tc.tile_pool(name="C", bufs=2, space="PSUM"))

            u_all = up.tile([CAP, S * CAP], fp16)
            nc.sync.dma_start(u_all[:], u_ap[:])
            col_all = lp.tile([CAP, S * 12], fp16)
            nc.sync.dma_start(col_all[:], col_ap[:])
            # all basis+coef data SBUF-resident: one wide tile, 4 DMAs total
            bca_t = bp.tile([128, S * BCW], fp16)
            engines = [nc.sync, nc.scalar, nc.sync, nc.scalar]
            for j in range(4):
                engines[j].dma_start(bca_t[GRP * j:GRP * j + 6, :], bc_ap[j])

            ostage = None
            for s in range(S):
                o = s * BCW
                z_t = zp.tile([128, NPIX], fp32)
                for j in range(4):
                    for h in range(2):
                        for pp in range(2):  # coef hi then lo, accumulated
                            nc.tensor.matmul(
                                z_t[GRP * j:GRP * (j + 1),
                                    h * 512:(h + 1) * 512],
                                bca_t[GRP * j:GRP * j + 6,
                                      o + NPIX + GRP * pp:
                                      o + NPIX + GRP * (pp + 1)],
                                bca_t[GRP * j:GRP * j + 6,
                                      o + h * 512:o + (h + 1) * 512],
                                start=(pp == 0), stop=(pp == 1),
                                tile_position=(GRP * j, GRP * j))

                e_t = ep.tile([128, NPIX], fp32)
                nc.scalar.activation(e_t[:], z_t[:], Act.Exp)

                m_t = mp.tile([128, NPIX], fp32)
                nc.vector.tensor_scalar(m_t[:], e_t[:], 1.0 / 255.0, None,
                                        Alu.is_ge)

                al_t = ap_.tile([128, NPIX], fp32)
                nc.vector.scalar_tensor_tensor(al_t[:], e_t[:], 0.99, m_t[:],
                                               Alu.min, Alu.mult)

                s_t = sp.tile([128, NPIX], fp16)
                nc.scalar.activation(s_t[:], al_t[:], Act.Ln, bias=1.0,
                                     scale=-1.0)

                S_t = Sp.tile([128, NPIX], fp32)
                for h in range(2):
                    nc.tensor.matmul(S_t[:, h * 512:(h + 1) * 512],
                                     u_all[:, s * CAP:(s + 1) * CAP],
                                     s_t[:, h * 512:(h + 1) * 512],
                                     start=True, stop=True)

                T_t = tp.tile([128, NPIX], fp32)
                nc.scalar.activation(T_t[:], S_t[:], Act.Exp)

                w_t = wp.tile([128, NPIX], fp16)
                nc.vector.tensor_tensor(w_t[:], al_t[:], T_t[:], Alu.mult)

                C_t = Cp.tile([128, 96], fp32)
                for jc in range(8):
                    nc.tensor.matmul(C_t[:, jc * 12:(jc + 1) * 12],
                                     w_t[:, jc * 128:(jc + 1) * 128],
                                     col_all[:, s * 12:(s + 1) * 12],
                                     start=True, stop=True)

                g = s % OB
                if g == 0:
                    ostage = cop.tile([128, OB * 96], fp32)
                nc.vector.tensor_copy(ostage[:, g * 96:(g + 1) * 96], C_t[:])
                if g == OB - 1 or s == S - 1:
                    s0 = s - g
                    nc.scalar.dma_start(
                        out_ap[:, s0 * 96:(s + 1) * 96],
                        ostage[:, :(g + 1) * 96])

    nc.compile()
    return nc


def _get_state(S):
    key = ("nc", S)
    if key not in _STATE:
        _STATE[key] = _build_module(S)
    return _STATE[key]


def _prepare_inputs(means_2d, covs_2d, depth_features, opacity_features,
                    color_features):
    """Host prep: sort, conic, per-block cull, superchunk packing.

    Returns (in_maps, S, block_map) where block_map[bidx] =
    (core, superchunk, j0) for every scheduled (non-empty) block.
    """
    order = np.argsort(depth_features[:, 0], kind="stable")
    m = means_2d[order].astype(np.float64)
    cv = covs_2d[order].astype(np.float64)
    op = opacity_features[order, 0].astype(np.float64)
    col = color_features[order].astype(np.float64)

    a, b, c = cv[:, 0], cv[:, 1], cv[:, 2]
    det = np.maximum(a * c - b * b, 1e-8)
    ia, ib, ic = c / det, -b / det, a / det

    alive = op * 255.0 >= 1.0 - 1e-6
    qsel = np.where(alive, 2.0 * np.log(np.maximum(255.0 * op, 1.0)), 0.0) + 0.3
    dx = np.sqrt(np.maximum(qsel * a, 0.0)) + 0.5
    dy = np.sqrt(np.maximum(qsel * c, 0.0)) + 0.5

    mx, my = m[:, 0], m[:, 1]
    bx0 = np.arange(NBX) * BW
    by0 = np.arange(NBY) * BH
    selx = (mx[:, None] + dx[:, None] >= bx0[None, :] + 0.5) & \
           (mx[:, None] - dx[:, None] <= bx0[None, :] + BW - 0.5)
    sely = (my[:, None] + dy[:, None] >= by0[None, :] + 0.5) & \
           (my[:, None] - dy[:, None] <= by0[None, :] + BH - 0.5)
    sel = selx[:, None, :] & sely[:, :, None] & alive[:, None, None]

    # block lists (depth order preserved: np.nonzero is ascending)
    blocks = []  # (bidx, idx array, ngroups)
    for byi in range(NBY):
        for bxi in range(NBX):
            bidx = byi * NBX + bxi
            idx = np.nonzero(sel[:, byi, bxi])[0]
            L = idx.size
            if L == 0:
                continue
            if L > CAP:
                raise RuntimeError(f"block {bidx}: {L} gaussians > {CAP}")
            blocks.append((bidx, idx, (L + GRP - 1) // GRP))

    # assign blocks to cores balancing total group count
    blocks.sort(key=lambda t: (-t[2], -t[1].size))
    core_groups = [0] * NCORES
    core_blocks = [[] for _ in range(NCORES)]
    for blk in blocks:
        ci = min(range(NCORES), key=lambda cc: core_groups[cc])
        core_blocks[ci].append(blk)
        core_groups[ci] += blk[2]

    # pack each core's blocks into superchunks (first-fit decreasing;
    # each block occupies ng consecutive groups of one superchunk)
    core_scs = []
    for ci in range(NCORES):
        scs = []   # each: list of (bidx, idx, j0, ng)
        free = []  # free groups per superchunk
        for bidx, idx, ng in core_blocks[ci]:
            for si, fr in enumerate(free):
                if fr >= ng:
                    j0 = 4 - fr
                    scs[si].append((bidx, idx, j0, ng))
                    free[si] -= ng
                    break
            else:
                scs.append([(bidx, idx, 0, ng)])
                free.append(4 - ng)
        core_scs.append(scs)

    S = max(len(scs) for scs in core_scs)

    # packed arrays
    ixl = np.arange(BW, dtype=np.float64) + 0.5 - BW / 2
    iyl = np.arange(BH, dtype=np.float64) + 0.5 - BH / 2
    Xl = np.tile(ixl, BH)               # pixel p = iy*BW + ix
    Yl = np.repeat(iyl, BW)
    basis_block = np.stack(
        [np.ones(NPIX), Xl, Yl, Xl * Xl, Xl * Yl, Yl * Yl]).astype(np.float16)
    basis_dummy = np.zeros((6, NPIX), np.float16)
    basis_dummy[0] = 1.0

    in_maps = []
    block_map = {}
    for ci in range(NCORES):
        bc = np.zeros((S, 4, 6, BCW), np.float16)
        bc[:, :, :, :NPIX] = basis_dummy[None, None]
        bc[:, :, 0, NPIX:NPIX + GRP] = -30000.0
        colbd = np.zeros((CAP, S, 12), np.float16)
        u = np.zeros((CAP, S, CAP), np.float16)
        for si, sc in enumerate(core_scs[ci]):
            for bidx, idx, j0, ng in sc:
                byi, bxi = divmod(bidx, NBX)
                cx = bx0[bxi] + BW / 2
                cy = by0[byi] + BH / 2
                L = idx.size
                mxp = mx[idx] - cx
                myp = my[idx] - cy
                cf = np.zeros((6, L))
                cf[0] = (-0.5 * ia[idx] * mxp * mxp - ib[idx] * mxp * myp
                         - 0.5 * ic[idx] * myp * myp + np.log(op[idx]))
                cf[1] = ia[idx] * mxp + ib[idx] * myp
                cf[2] = ib[idx] * mxp + ic[idx] * myp
                cf[3] = -0.5 * ia[idx]
                cf[4] = -ib[idx]
                cf[5] = -0.5 * ic[idx]
                cf = cf.astype(np.float32)
                cf_hi = cf.astype(np.float16)
                cf_lo = (cf - cf_hi.astype(np.float32)).astype(np.float16)
                for li in range(ng):
                    j = j0 + li
                    lo = li * GRP
                    n = min(GRP, L - lo)
                    bc[si, j, :, :NPIX] = basis_block
                    bc[si, j, :, NPIX:NPIX + n] = cf_hi[:, lo:lo + n]
                    bc[si, j, :, NPIX + GRP:NPIX + GRP + n] = \
                        cf_lo[:, lo:lo + n]
                r0, r1 = GRP * j0, GRP * j0 + L
                colbd[r0:r1, si, 3 * j0:3 * j0 + 3] = \
                    col[idx].astype(np.float16)
                u[r0:r1, si, r0:r1] = np.triu(np.ones((L, L), np.float16), 1)
                block_map[bidx] = (ci, si, j0)
        in_maps.append({
            "bc": np.ascontiguousarray(
                bc.transpose(1, 2, 0, 3).reshape(4, 6, S * BCW)),
            "colors": np.ascontiguousarray(colbd.reshape(CAP, S * 12)),
            "u": np.ascontiguousarray(u.reshape(CAP, S * CAP)),
        })
    return in_maps, S, block_map


def _unshard(results, S, block_map):
    out = np.zeros((3, H, W), np.float32)
    for bidx, (ci, si, j0) in block_map.items():
        byi, bxi = divmod(bidx, NBX)
        Cc = results[ci]["outC"]  # [128, S*96]
        blk = Cc[:, si * 96:(si + 1) * 96].reshape(128, 8, 12)
        # C[ch, 128*jc + q] = blk[q, jc, 3*j0 + ch]
        cb = blk[:, :, 3 * j0:3 * j0 + 3].transpose(2, 1, 0).reshape(3, NPIX)
        out[:, byi * BH:(byi + 1) * BH, bxi * BW:(bxi + 1) * BW] = \
            cb.reshape(3, BH, BW)
    return out


def kernel(means_2d, covs_2d, depth_features, opacity_features,
           color_features, screen_space_points=None, width=W, height=H,
           **_unused):
    from concourse.bass_utils import run_bass_kernel_spmd

    in_maps, S, block_map = _prepare_inputs(
        np.asarray(means_2d), np.asarray(covs_2d),
        np.asarray(depth_features), np.asarray(opacity_features),
        np.asarray(color_features))
    nc = _get_state(S)
    res = run_bass_kernel_spmd(nc, in_maps, core_ids=list(range(NCORES)))
    return _unshard(res.results, S, block_map)


# revision 15
# speedup vs baseline: 7.6936x; 1.2366x over previous
"""Differentiable 3DGS tile rasterizer forward pass on 8 Trainium2 NeuronCores.

Strategy (sharding_hint: shard pixels, replicate gaussian params):
  Host: depth-sort gaussians, compute conic + per-block (32x32 px) polynomial
  coefficients, cull per block on the alpha >= 1/255 support, then pack
  blocks into 128-row "superchunks": the 128 PE partitions are split into
  four 32-row groups, each group holding (a slice of) one block's gaussian
  list. One block may span 1..4 consecutive groups (a "run").

  Device (SPMD over 8 cores, S superchunks each), per superchunk:
    z[g, p]  = coef_g . basis_p     4x2 packed tile_position matmuls (fp32)
    e        = exp(z)               ScalarE          == op*exp(power)
    m        = e >= 1/255           VectorE
    alpha    = min(e, 0.99) * m     VectorE fused stt
    s        = ln(1 - alpha)        ScalarE, fp16 out
    S[g, p]  = sum_{k<g, same run} s[k, p]   per-superchunk triangular matmul
    T        = exp(S)               ScalarE   exclusive transmittance
    w        = alpha * T            VectorE, fp16 out
    C[q, 12*jc + 3*j0 + c] = sum_g w[g, 128*jc + q] col_bd[g, .]  (matmul)
  Host: scatter per-(superchunk, run) C back into the [3, H, W] image.
"""

import sys

sys.path.insert(0, "/opt/trn_rl_repo")

import numpy as np

P, H, W = 2048, 512, 512
BW = BH = 32                      # pixel block size
NBX, NBY = W // BW, H // BH       # 16 x 16 blocks
NBLOCKS = NBX * NBY               # 256
NCORES = 8
NPIX = BW * BH                    # 1024 pixels per block
CAP = 128                         # max gaussians per block (4 groups x 32)
GRP = 32                          # rows per group
BCW = NPIX + 2 * GRP              # packed [basis | coef_hi | coef_lo] per group
OB = 4                            # superchunks per output DMA batch

_STATE = {}


def _patch_act_tables():
    """Make Exp/Ln resolve only to the combined natural_log_exp_and_others
    table set, so the act-table-load pass emits one load instead of
    alternating ~2.7us set switches between every Exp and Ln activation."""
    from concourse import bacc, mybir, hw_specs

    if getattr(bacc, "_act_tables_patched", False):
        return
    orig = hw_specs.get_activation_tables
    both = {mybir.ActivationFunctionType.Exp, mybir.ActivationFunctionType.Ln}

    def patched(arch):
        tabs = dict(orig(arch))
        return {name: (fns if name == "natural_log_exp_and_others"
                       else set(fns) - both)
                for name, fns in tabs.items()}

    hw_specs.get_activation_tables = patched
    bacc.get_activation_tables = patched
    bacc._act_tables_patched = True


def _build_module(S):
    import concourse.tile as tile
    from concourse import bacc, mybir
    from contextlib import ExitStack

    _patch_act_tables()

    fp32 = mybir.dt.float32
    fp16 = mybir.dt.float16
    Act = mybir.ActivationFunctionType
    Alu = mybir.AluOpType

    nc = bacc.Bacc("TRN2", target_bir_lowering=False, debug=False,
                   num_devices=NCORES)

    bc_ap = nc.dram_tensor("bc", [4, 6, S * BCW], fp16,
                           kind="ExternalInput").ap()
    col_ap = nc.dram_tensor("colors", [CAP, S * 12], fp16,
                            kind="ExternalInput").ap()
    u_ap = nc.dram_tensor("u", [CAP, S * CAP], fp16,
                          kind="ExternalInput").ap()
    out_ap = nc.dram_tensor("outC", [128, S * 96], fp32,
                            kind="ExternalOutput").ap()

    with tile.TileContext(nc) as tc:
        with ExitStack() as ctx:
            up = ctx.enter_context(tc.tile_pool(name="u", bufs=1))
            lp = ctx.enter_context(tc.tile_pool(name="col", bufs=1))
            bp = ctx.enter_context(tc.tile_pool(name="bc", bufs=1))
            ep = ctx.enter_context(tc.tile_pool(name="e", bufs=2))
            mp = ctx.enter_context(tc.tile_pool(name="m", bufs=2))
            ap_ = ctx.enter_context(tc.tile_pool(name="alpha", bufs=2))
            sp = ctx.enter_context(tc.tile_pool(name="s", bufs=2))
            tp = ctx.enter_context(tc.tile_pool(name="t", bufs=2))
            wp = ctx.enter_context(tc.tile_pool(name="w", bufs=2))
            cop = ctx.enter_context(tc.tile_pool(name="cout", bufs=2))
            zp = ctx.enter_context(tc.tile_pool(name="z", bufs=2, space="PSUM"))
            Sp = ctx.enter_context(tc.tile_pool(name="S", bufs=1, space="PSUM"))
            Cp = ctx.enter_context(tc.tile_pool(name="C", bufs=2, space="PSUM"))

            u_all = up.tile([CAP, S * CAP], fp16)
            nc.sync.dma_start(u_all[:], u_ap[:])
            col_all = lp.tile([CAP, S * 12], fp16)
            nc.sync.dma_start(col_all[:], col_ap[:])
            # all basis+coef data SBUF-resident: one wide tile, 4 DMAs total
            bca_t = bp.tile([128, S * BCW], fp16)
            engines = [nc.sync, nc.scalar, nc.sync, nc.scalar]
            for j in range(4):
                engines[j].dma_start(bca_t[GRP * j:GRP * j + 6, :], bc_ap[j])

            # 3-stage software pipeline across superchunks so no engine's
            # strict-FIFO queue waits on a same-superchunk cross-engine
            # chain: emit colors(s-2) | alpha-frontend(s) | scan/T/w(s-1).
            ost = {"t": None}

            def frontend(s):
                o = s * BCW
                z_t = zp.tile([128, NPIX], fp32)
                for j in range(4):
                    for h in range(2):
                        for pp in range(2):  # coef hi then lo, accumulated
                            nc.tensor.matmul(
                                z_t[GRP * j:GRP * (j + 1),
                                    h * 512:(h + 1) * 512],
                                bca_t[GRP * j:GRP * j + 6,
                                      o + NPIX + GRP * pp:
                                      o + NPIX + GRP * (pp + 1)],
                                bca_t[GRP * j:GRP * j + 6,
                                      o + h * 512:o + (h + 1) * 512],
                                start=(pp == 0), stop=(pp == 1),
                                tile_position=(GRP * j, GRP * j))
                e_t = ep.tile([128, NPIX], fp32)
                nc.scalar.activation(e_t[:], z_t[:], Act.Exp)
                m_t = mp.tile([128, NPIX], fp32)
                nc.vector.tensor_scalar(m_t[:], e_t[:], 1.0 / 255.0, None,
                                        Alu.is_ge)
                al_t = ap_.tile([128, NPIX], fp32)
                nc.vector.scalar_tensor_tensor(al_t[:], e_t[:], 0.99, m_t[:],
                                               Alu.min, Alu.mult)
                s_t = sp.tile([128, NPIX], fp16)
                nc.scalar.activation(s_t[:], al_t[:], Act.Ln, bias=1.0,
                                     scale=-1.0)
                return {"s": s, "s_t": s_t, "al": al_t}

            def mid(st):
                s = st["s"]
                S_t = Sp.tile([128, NPIX], fp32)
                for h in range(2):
                    nc.tensor.matmul(S_t[:, h * 512:(h + 1) * 512],
                                     u_all[:, s * CAP:(s + 1) * CAP],
                                     st["s_t"][:, h * 512:(h + 1) * 512],
                                     start=True, stop=True)
                T_t = tp.tile([128, NPIX], fp32)
                nc.scalar.activation(T_t[:], S_t[:], Act.Exp)
                w_t = wp.tile([128, NPIX], fp16)
                nc.vector.tensor_tensor(w_t[:], st["al"][:], T_t[:], Alu.mult)
                st["w"] = w_t

            def back(st):
                s = st["s"]
                C_t = Cp.tile([128, 96], fp32)
                for jc in range(8):
                    nc.tensor.matmul(C_t[:, jc * 12:(jc + 1) * 12],
                                     st["w"][:, jc * 128:(jc + 1) * 128],
                                     col_all[:, s * 12:(s + 1) * 12],
                                     start=True, stop=True)
                g = s % OB
                if g == 0:
                    ost["t"] = cop.tile([128, OB * 96], fp32, name="ostage", tag="ostage")
                nc.vector.tensor_copy(ost["t"][:, g * 96:(g + 1) * 96],
                                      C_t[:])
                if g == OB - 1 or s == S - 1:
                    s0 = s - g
                    nc.scalar.dma_start(out_ap[:, s0 * 96:(s + 1) * 96],
                                        ost["t"][:, :(g + 1) * 96])

            pipe = {}
            for step in range(S + 2):
                if step >= 2:
                    back(pipe.pop(step - 2))
                if step < S:
                    pipe[step] = frontend(step)
                if 0 <= step - 1 < S:
                    mid(pipe[step - 1])

    nc.compile()
    return nc


def _get_state(S):
    key = ("nc", S)
    if key not in _STATE:
        _STATE[key] = _build_module(S)
    return _STATE[key]


def _prepare_inputs(means_2d, covs_2d, depth_features, opacity_features,
                    color_features):
    """Host prep: sort, conic, per-block cull, superchunk packing.

    Returns (in_maps, S, block_map) where block_map[bidx] =
    (core, superchunk, j0) for every scheduled (non-empty) block.
    """
    order = np.argsort(depth_features[:, 0], kind="stable")
    m = means_2d[order].astype(np.float64)
    cv = covs_2d[order].astype(np.float64)
    op = opacity_features[order, 0].astype(np.float64)
    col = color_features[order].astype(np.float64)

    a, b, c = cv[:, 0], cv[:, 1], cv[:, 2]
    det = np.maximum(a * c - b * b, 1e-8)
    ia, ib, ic = c / det, -b / det, a / det

    alive = op * 255.0 >= 1.0 - 1e-6
    qsel = np.where(alive, 2.0 * np.log(np.maximum(255.0 * op, 1.0)), 0.0) + 0.3
    dx = np.sqrt(np.maximum(qsel * a, 0.0)) + 0.5
    dy = np.sqrt(np.maximum(qsel * c, 0.0)) + 0.5

    mx, my = m[:, 0], m[:, 1]
    bx0 = np.arange(NBX) * BW
    by0 = np.arange(NBY) * BH
    selx = (mx[:, None] + dx[:, None] >= bx0[None, :] + 0.5) & \
           (mx[:, None] - dx[:, None] <= bx0[None, :] + BW - 0.5)
    sely = (my[:, None] + dy[:, None] >= by0[None, :] + 0.5) & \
           (my[:, None] - dy[:, None] <= by0[None, :] + BH - 0.5)
    sel = selx[:, None, :] & sely[:, :, None] & alive[:, None, None]

    # block lists (depth order preserved: np.nonzero is ascending)
    blocks = []  # (bidx, idx array, ngroups)
    for byi in range(NBY):
        for bxi in range(NBX):
            bidx = byi * NBX + bxi
            idx = np.nonzero(sel[:, byi, bxi])[0]
            L = idx.size
            if L == 0:
                continue
            if L > CAP:
                raise RuntimeError(f"block {bidx}: {L} gaussians > {CAP}")
            blocks.append((bidx, idx, (L + GRP - 1) // GRP))

    # assign blocks to cores balancing total group count
    blocks.sort(key=lambda t: (-t[2], -t[1].size))
    core_groups = [0] * NCORES
    core_blocks = [[] for _ in range(NCORES)]
    for blk in blocks:
        ci = min(range(NCORES), key=lambda cc: core_groups[cc])
        core_blocks[ci].append(blk)
        core_groups[ci] += blk[2]

    # pack each core's blocks into superchunks (first-fit decreasing;
    # each block occupies ng consecutive groups of one superchunk)
    core_scs = []
    for ci in range(NCORES):
        scs = []   # each: list of (bidx, idx, j0, ng)
        free = []  # free groups per superchunk
        for bidx, idx, ng in core_blocks[ci]:
            for si, fr in enumerate(free):
                if fr >= ng:
                    j0 = 4 - fr
                    scs[si].append((bidx, idx, j0, ng))
                    free[si] -= ng
                    break
            else:
                scs.append([(bidx, idx, 0, ng)])
                free.append(4 - ng)
        core_scs.append(scs)

    S = max(len(scs) for scs in core_scs)

    # packed arrays
    ixl = np.arange(BW, dtype=np.float64) + 0.5 - BW / 2
    iyl = np.arange(BH, dtype=np.float64) + 0.5 - BH / 2
    Xl = np.tile(ixl, BH)               # pixel p = iy*BW + ix
    Yl = np.repeat(iyl, BW)
    basis_block = np.stack(
        [np.ones(NPIX), Xl, Yl, Xl * Xl, Xl * Yl, Yl * Yl]).astype(np.float16)
    basis_dummy = np.zeros((6, NPIX), np.float16)
    basis_dummy[0] = 1.0

    in_maps = []
    block_map = {}
    for ci in range(NCORES):
        bc = np.zeros((S, 4, 6, BCW), np.float16)
        bc[:, :, :, :NPIX] = basis_dummy[None, None]
        bc[:, :, 0, NPIX:NPIX + GRP] = -30000.0
        colbd = np.zeros((CAP, S, 12), np.float16)
        u = np.zeros((CAP, S, CAP), np.float16)
        for si, sc in enumerate(core_scs[ci]):
            for bidx, idx, j0, ng in sc:
                byi, bxi = divmod(bidx, NBX)
                cx = bx0[bxi] + BW / 2
                cy = by0[byi] + BH / 2
                L = idx.size
                mxp = mx[idx] - cx
                myp = my[idx] - cy
                cf = np.zeros((6, L))
                cf[0] = (-0.5 * ia[idx] * mxp * mxp - ib[idx] * mxp * myp
                         - 0.5 * ic[idx] * myp * myp + np.log(op[idx]))
                cf[1] = ia[idx] * mxp + ib[idx] * myp
                cf[2] = ib[idx] * mxp + ic[idx] * myp
                cf[3] = -0.5 * ia[idx]
                cf[4] = -ib[idx]
                cf[5] = -0.5 * ic[idx]
                cf = cf.astype(np.float32)
                cf_hi = cf.astype(np.float16)
                cf_lo = (cf - cf_hi.astype(np.float32)).astype(np.float16)
                for li in range(ng):
                    j = j0 + li
                    lo = li * GRP
                    n = min(GRP, L - lo)
                    bc[si, j, :, :NPIX] = basis_block
                    bc[si, j, :, NPIX:NPIX + n] = cf_hi[:, lo:lo + n]
                    bc[si, j, :, NPIX + GRP:NPIX + GRP + n] = \
                        cf_lo[:, lo:lo + n]
                r0, r1 = GRP * j0, GRP * j0 + L
                colbd[r0:r1, si, 3 * j0:3 * j0 + 3] = \
                    col[idx].astype(np.float16)
                u[r0:r1, si, r0:r1] = np.triu(np.ones((L, L), np.float16), 1)
                block_map[bidx] = (ci, si, j0)
        in_maps.append({
            "bc": np.ascontiguousarray(
                bc.transpose(1, 2, 0, 3).reshape(4, 6, S * BCW)),
            "colors": np.ascontiguousarray(colbd.reshape(CAP, S * 12)),
            "u": np.ascontiguousarray(u.reshape(CAP, S * CAP)),
        })
    return in_maps, S, block_map


def _unshard(results, S, block_map):
    out = np.zeros((3, H, W), np.float32)
    for bidx, (ci, si, j0) in block_map.items():
        byi, bxi = divmod(bidx, NBX)
        Cc = results[ci]["outC"]  # [128, S*96]
        blk = Cc[:, si * 96:(si + 1) * 96].reshape(128, 8, 12)
        # C[ch, 128*jc + q] = blk[q, jc, 3*j0 + ch]
        cb = blk[:, :, 3 * j0:3 * j0 + 3].transpose(2, 1, 0).reshape(3, NPIX)
        out[:, byi * BH:(byi + 1) * BH, bxi * BW:(bxi + 1) * BW] = \
            cb.reshape(3, BH, BW)
    return out


def kernel(means_2d, covs_2d, depth_features, opacity_features,
           color_features, screen_space_points=None, width=W, height=H,
           **_unused):
    from concourse.bass_utils import run_bass_kernel_spmd

    in_maps, S, block_map = _prepare_inputs(
        np.asarray(means_2d), np.asarray(covs_2d),
        np.asarray(depth_features), np.asarray(opacity_features),
        np.asarray(color_features))
    nc = _get_state(S)
    res = run_bass_kernel_spmd(nc, in_maps, core_ids=list(range(NCORES)))
    return _unshard(res.results, S, block_map)


# revision 16
# speedup vs baseline: 9.2548x; 1.2029x over previous
"""Differentiable 3DGS tile rasterizer forward pass on 8 Trainium2 NeuronCores.

Strategy (sharding_hint: shard pixels, replicate gaussian params):
  Host: depth-sort gaussians, compute conic + per-block (32x32 px) polynomial
  coefficients, cull per block on the alpha >= 1/255 support, then pack
  blocks into 128-row "superchunks": the 128 PE partitions are split into
  four 32-row groups, each group holding (a slice of) one block's gaussian
  list. One block may span 1..4 consecutive groups (a "run").

  Device (SPMD over 8 cores, S superchunks each), per superchunk:
    z[g, p]  = coef_g . basis_p     4x2 packed tile_position matmuls (fp32)
    e        = exp(z)               ScalarE          == op*exp(power)
    m        = e >= 1/255           VectorE
    alpha    = min(e, 0.99) * m     VectorE fused stt
    s        = ln(1 - alpha)        ScalarE, fp16 out
    S[g, p]  = sum_{k<g, same run} s[k, p]   per-superchunk triangular matmul
    T        = exp(S)               ScalarE   exclusive transmittance
    w        = alpha * T            VectorE, fp16 out
    C[q, 12*jc + 3*j0 + c] = sum_g w[g, 128*jc + q] col_bd[g, .]  (matmul)
  Host: scatter per-(superchunk, run) C back into the [3, H, W] image.
"""

import sys

sys.path.insert(0, "/opt/trn_rl_repo")

import numpy as np

P, H, W = 2048, 512, 512
BW = BH = 32                      # pixel block size
NBX, NBY = W // BW, H // BH       # 16 x 16 blocks
NBLOCKS = NBX * NBY               # 256
NCORES = 8
NPIX = BW * BH                    # 1024 pixels per block
CAP = 128                         # max gaussians per block (4 groups x 32)
GRP = 32                          # rows per group
BCW = NPIX + 2 * GRP              # packed [basis | coef_hi | coef_lo] per group
OB = 4                            # superchunks per output DMA batch

_STATE = {}


def _patch_act_tables():
    """Make Exp/Ln resolve only to the combined natural_log_exp_and_others
    table set, so the act-table-load pass emits one load instead of
    alternating ~2.7us set switches between every Exp and Ln activation."""
    from concourse import bacc, mybir, hw_specs

    if getattr(bacc, "_act_tables_patched", False):
        return
    orig = hw_specs.get_activation_tables
    both = {mybir.ActivationFunctionType.Exp, mybir.ActivationFunctionType.Ln}

    def patched(arch):
        tabs = dict(orig(arch))
        return {name: (fns if name == "natural_log_exp_and_others"
                       else set(fns) - both)
                for name, fns in tabs.items()}

    hw_specs.get_activation_tables = patched
    bacc.get_activation_tables = patched
    bacc._act_tables_patched = True


def _build_module(S, loop_R=None):
    import concourse.tile as tile
    from concourse import bacc, mybir
    from contextlib import ExitStack

    _patch_act_tables()

    fp32 = mybir.dt.float32
    fp16 = mybir.dt.float16
    Act = mybir.ActivationFunctionType
    Alu = mybir.AluOpType

    nc = bacc.Bacc("TRN2", target_bir_lowering=False, debug=False,
                   num_devices=NCORES)

    bc_ap = nc.dram_tensor("bc", [4, 6, S * BCW], fp16,
                           kind="ExternalInput").ap()
    col_ap = nc.dram_tensor("colors", [CAP, S * 12], fp16,
                            kind="ExternalInput").ap()
    u_ap = nc.dram_tensor("u", [CAP, S * CAP], fp16,
                          kind="ExternalInput").ap()
    out_ap = nc.dram_tensor("outC", [128, S * 96], fp32,
                            kind="ExternalOutput").ap()

    with tile.TileContext(nc) as tc:
        with ExitStack() as ctx:
            up = ctx.enter_context(tc.tile_pool(name="u", bufs=1))
            lp = ctx.enter_context(tc.tile_pool(name="col", bufs=1))
            bp = ctx.enter_context(tc.tile_pool(name="bc", bufs=1))
            ep = ctx.enter_context(tc.tile_pool(name="e", bufs=2))
            mp = ctx.enter_context(tc.tile_pool(name="m", bufs=2))
            ap_ = ctx.enter_context(tc.tile_pool(name="alpha", bufs=2))
            sp = ctx.enter_context(tc.tile_pool(name="s", bufs=2))
            tp = ctx.enter_context(tc.tile_pool(name="t", bufs=2))
            wp = ctx.enter_context(tc.tile_pool(name="w", bufs=2))
            cop = ctx.enter_context(tc.tile_pool(name="cout", bufs=2))
            zp = ctx.enter_context(tc.tile_pool(name="z", bufs=2, space="PSUM"))
            Sp = ctx.enter_context(tc.tile_pool(name="S", bufs=1, space="PSUM"))
            Cp = ctx.enter_context(tc.tile_pool(name="C", bufs=2, space="PSUM"))

            u_all = up.tile([CAP, S * CAP], fp16)
            nc.sync.dma_start(u_all[:], u_ap[:])
            col_all = lp.tile([CAP, S * 12], fp16)
            nc.sync.dma_start(col_all[:], col_ap[:])
            # all basis+coef data SBUF-resident: one wide tile, 4 DMAs total
            bca_t = bp.tile([128, S * BCW], fp16)
            engines = [nc.sync, nc.scalar, nc.sync, nc.scalar]
            for j in range(4):
                engines[j].dma_start(bca_t[GRP * j:GRP * j + 6, :], bc_ap[j])

            # 3-stage software pipeline across superchunks so no engine's
            # strict-FIFO queue waits on a same-superchunk cross-engine
            # chain: emit colors(s-2) | alpha-frontend(s) | scan/T/w(s-1).
            ost = {"t": None}

            def frontend(s):
                o = s * BCW
                z_t = zp.tile([128, NPIX], fp32)
                for j in range(4):
                    for h in range(2):
                        for pp in range(2):  # coef hi then lo, accumulated
                            nc.tensor.matmul(
                                z_t[GRP * j:GRP * (j + 1),
                                    h * 512:(h + 1) * 512],
                                bca_t[GRP * j:GRP * j + 6,
                                      o + NPIX + GRP * pp:
                                      o + NPIX + GRP * (pp + 1)],
                                bca_t[GRP * j:GRP * j + 6,
                                      o + h * 512:o + (h + 1) * 512],
                                start=(pp == 0), stop=(pp == 1),
                                tile_position=(GRP * j, GRP * j))
                e_t = ep.tile([128, NPIX], fp32)
                nc.scalar.activation(e_t[:], z_t[:], Act.Exp)
                m_t = mp.tile([128, NPIX], fp32)
                nc.vector.tensor_scalar(m_t[:], e_t[:], 1.0 / 255.0, None,
                                        Alu.is_ge)
                al_t = ap_.tile([128, NPIX], fp32)
                nc.vector.scalar_tensor_tensor(al_t[:], e_t[:], 0.99, m_t[:],
                                               Alu.min, Alu.mult)
                s_t = sp.tile([128, NPIX], fp16)
                nc.scalar.activation(s_t[:], al_t[:], Act.Ln, bias=1.0,
                                     scale=-1.0)
                return {"s": s, "s_t": s_t, "al": al_t}

            def mid(st):
                s = st["s"]
                S_t = Sp.tile([128, NPIX], fp32)
                for h in range(2):
                    nc.tensor.matmul(S_t[:, h * 512:(h + 1) * 512],
                                     u_all[:, s * CAP:(s + 1) * CAP],
                                     st["s_t"][:, h * 512:(h + 1) * 512],
                                     start=True, stop=True)
                T_t = tp.tile([128, NPIX], fp32)
                nc.scalar.activation(T_t[:], S_t[:], Act.Exp)
                w_t = wp.tile([128, NPIX], fp16)
                nc.vector.tensor_tensor(w_t[:], st["al"][:], T_t[:], Alu.mult)
                st["w"] = w_t

            def back(st):
                s = st["s"]
                C_t = Cp.tile([128, 96], fp32)
                for jc in range(8):
                    nc.tensor.matmul(C_t[:, jc * 12:(jc + 1) * 12],
                                     st["w"][:, jc * 128:(jc + 1) * 128],
                                     col_all[:, s * 12:(s + 1) * 12],
                                     start=True, stop=True)
                g = s % OB
                if g == 0:
                    ost["t"] = cop.tile([128, OB * 96], fp32, name="ostage", tag="ostage")
                nc.vector.tensor_copy(ost["t"][:, g * 96:(g + 1) * 96],
                                      C_t[:])
                if g == OB - 1 or s == S - 1:
                    s0 = s - g
                    nc.scalar.dma_start(out_ap[:, s0 * 96:(s + 1) * 96],
                                        ost["t"][:, :(g + 1) * 96])

            def run_pipeline():
                pipe = {}
                for step in range(S + 2):
                    if step >= 2:
                        back(pipe.pop(step - 2))
                    if step < S:
                        pipe[step] = frontend(step)
                    if 0 <= step - 1 < S:
                        mid(pipe[step - 1])

            if loop_R is None:
                run_pipeline()
            else:
                # repeat-loop variant used only for exec-time measurement
                with tc.For_i(0, loop_R, 1):
                    run_pipeline()

    nc.compile()
    return nc


def _get_state(S):
    key = ("nc", S)
    if key not in _STATE:
        _STATE[key] = _build_module(S)
    return _STATE[key]


def _prepare_inputs(means_2d, covs_2d, depth_features, opacity_features,
                    color_features):
    """Host prep: sort, conic, per-block cull, superchunk packing.

    Returns (in_maps, S, block_map) where block_map[bidx] =
    (core, superchunk, j0) for every scheduled (non-empty) block.
    """
    order = np.argsort(depth_features[:, 0], kind="stable")
    m = means_2d[order].astype(np.float64)
    cv = covs_2d[order].astype(np.float64)
    op = opacity_features[order, 0].astype(np.float64)
    col = color_features[order].astype(np.float64)

    a, b, c = cv[:, 0], cv[:, 1], cv[:, 2]
    det = np.maximum(a * c - b * b, 1e-8)
    ia, ib, ic = c / det, -b / det, a / det

    alive = op * 255.0 >= 1.0 - 1e-6
    qsel = np.where(alive, 2.0 * np.log(np.maximum(255.0 * op, 1.0)), 0.0) + 0.3
    dx = np.sqrt(np.maximum(qsel * a, 0.0)) + 0.5
    dy = np.sqrt(np.maximum(qsel * c, 0.0)) + 0.5

    mx, my = m[:, 0], m[:, 1]
    bx0 = np.arange(NBX) * BW
    by0 = np.arange(NBY) * BH
    selx = (mx[:, None] + dx[:, None] >= bx0[None, :] + 0.5) & \
           (mx[:, None] - dx[:, None] <= bx0[None, :] + BW - 0.5)
    sely = (my[:, None] + dy[:, None] >= by0[None, :] + 0.5) & \
           (my[:, None] - dy[:, None] <= by0[None, :] + BH - 0.5)
    sel = selx[:, None, :] & sely[:, :, None] & alive[:, None, None]

    # block lists (depth order preserved: np.nonzero is ascending)
    blocks = []  # (bidx, idx array, ngroups)
    for byi in range(NBY):
        for bxi in range(NBX):
            bidx = byi * NBX + bxi
            idx = np.nonzero(sel[:, byi, bxi])[0]
            L = idx.size
            if L == 0:
                continue
            if L > CAP:
                raise RuntimeError(f"block {bidx}: {L} gaussians > {CAP}")
            blocks.append((bidx, idx, (L + GRP - 1) // GRP))

    # assign blocks to cores balancing total group count
    blocks.sort(key=lambda t: (-t[2], -t[1].size))
    core_groups = [0] * NCORES
    core_blocks = [[] for _ in range(NCORES)]
    for blk in blocks:
        ci = min(range(NCORES), key=lambda cc: core_groups[cc])
        core_blocks[ci].append(blk)
        core_groups[ci] += blk[2]

    # pack each core's blocks into superchunks (first-fit decreasing;
    # each block occupies ng consecutive groups of one superchunk)
    core_scs = []
    for ci in range(NCORES):
        scs = []   # each: list of (bidx, idx, j0, ng)
        free = []  # free groups per superchunk
        for bidx, idx, ng in core_blocks[ci]:
            for si, fr in enumerate(free):
                if fr >= ng:
                    j0 = 4 - fr
                    scs[si].append((bidx, idx, j0, ng))
                    free[si] -= ng
                    break
            else:
                scs.append([(bidx, idx, 0, ng)])
                free.append(4 - ng)
        core_scs.append(scs)

    S = max(len(scs) for scs in core_scs)

    # packed arrays
    ixl = np.arange(BW, dtype=np.float64) + 0.5 - BW / 2
    iyl = np.arange(BH, dtype=np.float64) + 0.5 - BH / 2
    Xl = np.tile(ixl, BH)               # pixel p = iy*BW + ix
    Yl = np.repeat(iyl, BW)
    basis_block = np.stack(
        [np.ones(NPIX), Xl, Yl, Xl * Xl, Xl * Yl, Yl * Yl]).astype(np.float16)
    basis_dummy = np.zeros((6, NPIX), np.float16)
    basis_dummy[0] = 1.0

    in_maps = []
    block_map = {}
    for ci in range(NCORES):
        bc = np.zeros((S, 4, 6, BCW), np.float16)
        bc[:, :, :, :NPIX] = basis_dummy[None, None]
        bc[:, :, 0, NPIX:NPIX + GRP] = -30000.0
        colbd = np.zeros((CAP, S, 12), np.float16)
        u = np.zeros((CAP, S, CAP), np.float16)
        for si, sc in enumerate(core_scs[ci]):
            for bidx, idx, j0, ng in sc:
                byi, bxi = divmod(bidx, NBX)
                cx = bx0[bxi] + BW / 2
                cy = by0[byi] + BH / 2
                L = idx.size
                mxp = mx[idx] - cx
                myp = my[idx] - cy
                cf = np.zeros((6, L))
                cf[0] = (-0.5 * ia[idx] * mxp * mxp - ib[idx] * mxp * myp
                         - 0.5 * ic[idx] * myp * myp + np.log(op[idx]))
                cf[1] = ia[idx] * mxp + ib[idx] * myp
                cf[2] = ib[idx] * mxp + ic[idx] * myp
                cf[3] = -0.5 * ia[idx]
                cf[4] = -ib[idx]
                cf[5] = -0.5 * ic[idx]
                cf = cf.astype(np.float32)
                cf_hi = cf.astype(np.float16)
                cf_lo = (cf - cf_hi.astype(np.float32)).astype(np.float16)
                for li in range(ng):
                    j = j0 + li
                    lo = li * GRP
                    n = min(GRP, L - lo)
                    bc[si, j, :, :NPIX] = basis_block
                    bc[si, j, :, NPIX:NPIX + n] = cf_hi[:, lo:lo + n]
                    bc[si, j, :, NPIX + GRP:NPIX + GRP + n] = \
                        cf_lo[:, lo:lo + n]
                r0, r1 = GRP * j0, GRP * j0 + L
                colbd[r0:r1, si, 3 * j0:3 * j0 + 3] = \
                    col[idx].astype(np.float16)
                u[r0:r1, si, r0:r1] = np.triu(np.ones((L, L), np.float16), 1)
                block_map[bidx] = (ci, si, j0)
        in_maps.append({
            "bc": np.ascontiguousarray(
                bc.transpose(1, 2, 0, 3).reshape(4, 6, S * BCW)),
            "colors": np.ascontiguousarray(colbd.reshape(CAP, S * 12)),
            "u": np.ascontiguousarray(u.reshape(CAP, S * CAP)),
        })
    return in_maps, S, block_map


def _unshard(results, S, block_map):
    out = np.zeros((3, H, W), np.float32)
    for bidx, (ci, si, j0) in block_map.items():
        byi, bxi = divmod(bidx, NBX)
        Cc = results[ci]["outC"]  # [128, S*96]
        blk = Cc[:, si * 96:(si + 1) * 96].reshape(128, 8, 12)
        # C[ch, 128*jc + q] = blk[q, jc, 3*j0 + ch]
        cb = blk[:, :, 3 * j0:3 * j0 + 3].transpose(2, 1, 0).reshape(3, NPIX)
        out[:, byi * BH:(byi + 1) * BH, bxi * BW:(bxi + 1) * BW] = \
            cb.reshape(3, BH, BW)
    return out


def kernel(means_2d, covs_2d, depth_features, opacity_features,
           color_features, screen_space_points=None, width=W, height=H,
           **_unused):
    from concourse.bass_utils import run_bass_kernel_spmd

    in_maps, S, block_map = _prepare_inputs(
        np.asarray(means_2d), np.asarray(covs_2d),
        np.asarray(depth_features), np.asarray(opacity_features),
        np.asarray(color_features))
    nc = _get_state(S)
    res = run_bass_kernel_spmd(nc, in_maps, core_ids=list(range(NCORES)))
    return _unshard(res.results, S, block_map)
